# revision 1
# baseline (speedup 1.0000x reference)
"""Contextual-attention Trainium2 kernel (Bass/Tile), data-parallel over batch.

Math (per sequence b):
    Q = evo @ q_w.T + q_b                                  (L, 96)
    K = cat(evo, conv3(evo), conv5(evo)) @ k_w.T + k_b     (L, 96)
    V = plm @ v_w.T + v_b                                  (L, 96)
    P = softmax(Q K^T / sqrt(96), key-masked by seqlen)
    out = P @ V + V

Device-side reformulation (per core = one sequence):
  * The two convs + concat + K-projection fold into 5 shifted matmuls:
        K[l] = sum_{t=-2..2} evo[l+t] @ taps[t]  + bk      (host-folded weights)
  * Everything is computed transposed ([feature, L] layout) so the only
    contraction layouts needed are natural ones:
        QT = wqT.T @ evoT, KT = taps.T @ evoT(shifted), VT = wvT.T @ plmT
        ST[lk, lq] = KT_slice.T @ QT  -> exp via ACT with per-partition mask bias
        OT[0:96]   = sum_lk V1[lk].T @ ET[lk]   (V1 = [V | ones] natural layout,
        OT[96]     = softmax denominator         via on-chip PE transpose of VT)
  * All matmul operands are fp16 (PE streams 2B/cycle: fp32 is half rate), all
    accumulation is f32 in PSUM; exp runs in f32 on ScalarE. fp16 (not bf16)
    because every tensor here is O(1)-ranged and fp16 carries 3 more mantissa
    bits.
  * Key tiles entirely beyond max(seqlen) are skipped at build time; the
    per-core mask bias (0 / -1e6) zeroes partially-valid tiles exactly
    (exp(-1e6 + s) underflows to 0.0f, matching the reference's where()+softmax).
  * Final divide by denominator, +V residual, and the (96, L) -> (L, 96)
    transpose happen on host (tiny O(L*96) work).
"""

import os
import numpy as np

import concourse.bacc as bacc
import concourse.bass as bass
import concourse.tile as tile
from concourse import mybir
from concourse._compat import get_trn_type
from concourse.bass_utils import run_bass_kernel_spmd

B, L = 8, 2048
Q_IN, V_IN, QK, VD = 512, 1024, 96, 96
P = 128
NORM = float(1.0 / np.sqrt(QK))
F32 = mybir.dt.float32
F16 = mybir.dt.float16

LAST_EXEC_TIME_NS = None
LAST_RESULTS = None

_program_cache = {}


def _fold_k_weights(k_w, k_b, cn3_w, cn3_b, cn5_w, cn5_b):
    """K[l] = sum_{t in -2..2} evo[l+t] @ taps[t+2] + bk  (zero-padded shifts)."""
    A_evo = k_w[:, :Q_IN]
    A3 = k_w[:, Q_IN : Q_IN + VD]
    A5 = k_w[:, Q_IN + VD :]
    taps = np.zeros((5, Q_IN, QK), np.float32)
    for j in range(3):  # conv3 tap j acts at offset t = j-1
        taps[j - 1 + 2] += np.einsum("oc,cd->do", A3, cn3_w[:, :, j]).astype(np.float32)
    for j in range(5):  # conv5 tap j acts at offset t = j-2
        taps[j - 2 + 2] += np.einsum("oc,cd->do", A5, cn5_w[:, :, j]).astype(np.float32)
    taps[2] += A_evo.T
    bk = (k_b + A3 @ cn3_b + A5 @ cn5_b).astype(np.float32)
    return taps, bk


def _chunks(total, step=512):
    out = []
    o = 0
    while o < total:
        out.append((o, min(step, total - o)))
        o += step
    return out


def _build_program(nkt):
    """One SPMD program; all cores run NKT key tiles, masks differ per core."""
    lkw = nkt * P
    nc = bacc.Bacc(get_trn_type() or "TRN2", target_bir_lowering=False, debug=False)
    # weight/constant params (tiny, loaded first)
    wq = nc.declare_dram_parameter("wq", [P, 4 * QK], F16, isOutput=False)
    wk = nc.declare_dram_parameter("wk", [P, 20 * QK], F16, isOutput=False)
    wv = nc.declare_dram_parameter("wv", [P, 8 * QK], F16, isOutput=False)
    bqkv = nc.declare_dram_parameter("bqkv", [QK, 3], F32, isOutput=False)
    maskd = nc.declare_dram_parameter("mask", [P, nkt], F32, isOutput=False)
    identd = nc.declare_dram_parameter("ident", [P, P], F16, isOutput=False)
    # activations
    evoT = nc.declare_dram_parameter("evoT", [Q_IN, L + 4], F16, isOutput=False)
    plmT = nc.declare_dram_parameter("plmT", [V_IN, L], F16, isOutput=False)
    # outputs
    ot_out = nc.declare_dram_parameter("ot", [QK + 1, L], F32, isOutput=True)
    vt_out = nc.declare_dram_parameter("vt", [QK, L], F16, isOutput=True)

    add = mybir.AluOpType.add

    with tile.TileContext(nc) as tc:
        with tc.tile_pool(name="sing", bufs=1) as sing:
            # ---- weights + evo first (they gate the first matmuls); finer
            # partition splits engage more DMA engines in parallel ----
            wq_sb = sing.tile([P, 4, QK], F16, tag="wq")
            nc.sync.dma_start(out=wq_sb, in_=wq[:, :].rearrange("p (n o) -> p n o", o=QK))
            evo_sb = []
            for i in range(4):
                t = sing.tile([P, L + 4], F16, tag=f"evo{i}")
                for h in range(2):
                    nc.sync.dma_start(
                        out=t[h * 64 : (h + 1) * 64, :],
                        in_=evoT[i * P + h * 64 : i * P + (h + 1) * 64, :],
                    )
                evo_sb.append(t)
            wk_sb = sing.tile([P, 20, QK], F16, tag="wk")
            nc.sync.dma_start(out=wk_sb, in_=wk[:, :].rearrange("p (n o) -> p n o", o=QK))
            wv_sb = sing.tile([P, 8, QK], F16, tag="wv")
            nc.sync.dma_start(out=wv_sb, in_=wv[:, :].rearrange("p (n o) -> p n o", o=QK))
            b_sb = sing.tile([QK, 3], F32, tag="bqkv")
            nc.sync.dma_start(out=b_sb, in_=bqkv[:, :])
            mask_sb = sing.tile([P, nkt], F32, tag="mask")
            nc.sync.dma_start(out=mask_sb, in_=maskd[:, :])
            ident_sb = sing.tile([P, P], F16, tag="ident")
            nc.sync.dma_start(out=ident_sb, in_=identd[:, :])
            plm_sb = []
            for i in range(8):
                t = sing.tile([P, L], F16, tag=f"plm{i}")
                for h in range(2):
                    nc.sync.dma_start(
                        out=t[h * 64 : (h + 1) * 64, :],
                        in_=plmT[i * P + h * 64 : i * P + (h + 1) * 64, :],
                    )
                plm_sb.append(t)

            qt_sb = sing.tile([QK, L], F16, tag="qt")
            kt_sb = sing.tile([QK, lkw], F16, tag="kt")
            vt_sb = sing.tile([QK, L], F32, tag="vt")
            vt16_sb = sing.tile([QK, L], F16, tag="vt16")
            v1_sb = sing.tile([P, nkt, QK + 1], F16, tag="v1")
            ot_sb = sing.tile([QK + 1, L], F32, tag="ot")

            # ---- projections ----
            with (
                tc.tile_pool(name="proj_psum", bufs=3, space="PSUM") as proj_psum,
                tc.tile_pool(name="v1_psum", bufs=2, space="PSUM") as v1_psum,
            ):
                # QT = wq.T @ evoT  (+qb)
                for base, width in _chunks(L, 1024):
                    pt = proj_psum.tile([QK, 1024], F32, tag="proj")
                    for dt in range(4):
                        for o2, w2 in _chunks(width, 512):
                            nc.tensor.matmul(
                                pt[:, o2 : o2 + w2],
                                lhsT=wq_sb[:, dt, :],
                                rhs=evo_sb[dt][:, 2 + base + o2 : 2 + base + o2 + w2],
                                start=(dt == 0),
                                stop=(dt == 3),
                            )
                    nc.vector.tensor_scalar(
                        out=qt_sb[:, base : base + width],
                        in0=pt[:, :width],
                        scalar1=b_sb[:, 0:1],
                        scalar2=None,
                        op0=add,
                    )
                # KT = sum_t taps[t].T @ evoT(shift t-2)  (+kb), first lkw cols only
                for base, width in _chunks(lkw, 1024):
                    pt = proj_psum.tile([QK, 1024], F32, tag="proj")
                    n = 0
                    for t in range(5):
                        for dt in range(4):
                            for o2, w2 in _chunks(width, 512):
                                nc.tensor.matmul(
                                    pt[:, o2 : o2 + w2],
                                    lhsT=wk_sb[:, t * 4 + dt, :],
                                    rhs=evo_sb[dt][:, t + base + o2 : t + base + o2 + w2],
                                    start=(n == 0),
                                    stop=(n == 19),
                                )
                            n += 1
                    nc.vector.tensor_scalar(
                        out=kt_sb[:, base : base + width],
                        in0=pt[:, :width],
                        scalar1=b_sb[:, 1:2],
                        scalar2=None,
                        op0=add,
                    )
                # VT = wv.T @ plmT (+vb), full L (residual needs all of V)
                for base, width in _chunks(L, 1024):
                    pt = proj_psum.tile([QK, 1024], F32, tag="proj")
                    for dt in range(8):
                        for o2, w2 in _chunks(width, 512):
                            nc.tensor.matmul(
                                pt[:, o2 : o2 + w2],
                                lhsT=wv_sb[:, dt, :],
                                rhs=plm_sb[dt][:, base + o2 : base + o2 + w2],
                                start=(dt == 0),
                                stop=(dt == 7),
                            )
                    nc.vector.tensor_scalar(
                        out=vt_sb[:, base : base + width],
                        in0=pt[:, :width],
                        scalar1=b_sb[:, 2:3],
                        scalar2=None,
                        op0=add,
                    )
                    nc.scalar.copy(
                        out=vt16_sb[:, base : base + width],
                        in_=vt_sb[:, base : base + width],
                    )
                    nc.sync.dma_start(
                        out=vt_out[:, base : base + width],
                        in_=vt16_sb[:, base : base + width],
                    )

                # V1[j] = [V natural | ones]  via PE transpose of VT slices
                for j in range(nkt):
                    vp = v1_psum.tile([P, QK], F16, tag="v1p")
                    nc.tensor.transpose(
                        vp, vt16_sb[:, j * P : (j + 1) * P], ident_sb[:QK, :QK]
                    )
                    nc.vector.tensor_copy(out=v1_sb[:, j, :QK], in_=vp)
                    nc.vector.memset(v1_sb[:, j, QK : QK + 1], 1.0)

            # ---- attention (flash-style over l_q halves) ----
            with (
                tc.tile_pool(name="st_psum", bufs=3, space="PSUM") as st_psum,
                tc.tile_pool(name="ot_psum", bufs=1, space="PSUM") as ot_psum,
                tc.tile_pool(name="et", bufs=nkt + 2) as et_pool,
            ):
                for half in range(2):
                    hb = half * (L // 2)
                    ets = []
                    # scores + exp for the whole half (independent of V/plm)
                    for j in range(nkt):
                        stp = st_psum.tile([P, L // 2], F32, tag="stp")
                        for o2, w2 in _chunks(L // 2, 512):
                            nc.tensor.matmul(
                                stp[:, o2 : o2 + w2],
                                lhsT=kt_sb[:, j * P : (j + 1) * P],
                                rhs=qt_sb[:, hb + o2 : hb + o2 + w2],
                                start=True,
                                stop=True,
                            )
                        et = et_pool.tile([P, L // 2], F16, tag="et")
                        nc.scalar.activation(
                            out=et,
                            in_=stp,
                            func=mybir.ActivationFunctionType.Exp,
                            bias=mask_sb[:, j : j + 1],
                            scale=NORM,
                        )
                        ets.append(et)
                    # O^T accumulation (needs V1, i.e. plm)
                    otp = ot_psum.tile([QK + 1, L // 2], F32, tag="otp")
                    for j in range(nkt):
                        for o2, w2 in _chunks(L // 2, 512):
                            nc.tensor.matmul(
                                otp[:, o2 : o2 + w2],
                                lhsT=v1_sb[:, j, :],
                                rhs=ets[j][:, o2 : o2 + w2],
                                start=(j == 0),
                                stop=(j == nkt - 1),
                            )
                    # 97-partition DMAs defeat the DMA-engine fanout (must be a
                    # multiple of 16): store rows 0..95 and the denom row apart.
                    for o2, w2 in _chunks(L // 2, 512):
                        nc.vector.tensor_copy(
                            out=ot_sb[:, hb + o2 : hb + o2 + w2],
                            in_=otp[:, o2 : o2 + w2],
                        )
                        nc.sync.dma_start(
                            out=ot_out[:QK, hb + o2 : hb + o2 + w2],
                            in_=ot_sb[:QK, hb + o2 : hb + o2 + w2],
                        )
                        nc.scalar.dma_start(
                            out=ot_out[QK : QK + 1, hb + o2 : hb + o2 + w2],
                            in_=ot_sb[QK : QK + 1, hb + o2 : hb + o2 + w2],
                        )
    nc.finalize()
    return nc


def _prep_core_inputs(evo, plm, seqlen, weights, nkt):
    evoT = np.zeros((Q_IN, L + 4), np.float16)
    evoT[:, 2 : 2 + L] = evo.T
    plmT = np.ascontiguousarray(plm.T.astype(np.float16))
    j = np.arange(nkt)[None, :]
    p = np.arange(P)[:, None]
    mask = np.where(j * P + p < seqlen, 0.0, -1e6).astype(np.float32)
    m = {"evoT": evoT, "plmT": plmT, "mask": mask}
    m.update(weights)
    return m


def _pack_w(w, n):
    # (n*128, 96) f32 -> (128, n*96) f16 in the SBUF [p, n, o] layout
    return np.ascontiguousarray(
        w.reshape(n, P, QK).transpose(1, 0, 2).reshape(P, n * QK).astype(np.float16)
    )


def kernel(
    plm_embedding,
    evo_local,
    seqlengths,
    q_w,
    q_b,
    k_w,
    k_b,
    v_w,
    v_b,
    cn3_w,
    cn3_b,
    cn5_w,
    cn5_b,
):
    global LAST_EXEC_TIME_NS, LAST_RESULTS
    plm_embedding = np.asarray(plm_embedding, np.float32)
    evo_local = np.asarray(evo_local, np.float32)
    seqlengths = np.asarray(seqlengths)

    taps, bk = _fold_k_weights(
        np.asarray(k_w, np.float32),
        np.asarray(k_b, np.float32),
        np.asarray(cn3_w, np.float32),
        np.asarray(cn3_b, np.float32),
        np.asarray(cn5_w, np.float32),
        np.asarray(cn5_b, np.float32),
    )
    nkt = int(min(L // P, (int(seqlengths.max()) + P - 1) // P))
    bqkv = np.stack(
        [np.asarray(q_b, np.float32), bk, np.asarray(v_b, np.float32)], axis=1
    ).astype(np.float32)
    weights = {
        "wq": _pack_w(np.ascontiguousarray(np.asarray(q_w, np.float32).T), 4),
        "wk": _pack_w(taps.reshape(5 * Q_IN, QK), 20),
        "wv": _pack_w(np.ascontiguousarray(np.asarray(v_w, np.float32).T), 8),
        "bqkv": np.ascontiguousarray(bqkv),
        "ident": np.eye(P, dtype=np.float16),
    }

    if nkt not in _program_cache:
        _program_cache[nkt] = _build_program(nkt)
    nc = _program_cache[nkt]

    in_maps = [
        _prep_core_inputs(evo_local[b], plm_embedding[b], int(seqlengths[b]), weights, nkt)
        for b in range(B)
    ]
    trace = bool(os.environ.get("KBENCH_TRACE"))
    res = run_bass_kernel_spmd(nc, in_maps, list(range(B)), trace=trace)
    LAST_EXEC_TIME_NS = res.exec_time_ns
    LAST_RESULTS = res

    out = np.empty((B, L, VD), np.float32)
    for b in range(B):
        ot = res.results[b]["ot"]
        vt = res.results[b]["vt"]
        out[b] = (ot[:QK] / ot[QK : QK + 1]).T + vt.T
    return out



# revision 6
# speedup vs baseline: 1.0754x; 1.0754x over previous
"""Contextual-attention Trainium2 kernel (Bass/Tile), work-balanced across cores.

Math (per sequence b):
    Q = evo @ q_w.T + q_b                                  (L, 96)
    K = cat(evo, conv3(evo), conv5(evo)) @ k_w.T + k_b     (L, 96)
    V = plm @ v_w.T + v_b                                  (L, 96)
    P = softmax(Q K^T / sqrt(96), key-masked by seqlen)
    out = P @ V + V

Key facts driving the design (measured on hw):
  * PE streams 1 output column/cycle (~1.35 GHz) regardless of dtype; fp8
    DoubleRow processes 2 k-tiles per pass -> 2x for pairable contractions.
  * Work per sequence scales with nkt_b = ceil(seqlen_b/128) key tiles;
    per-batch nkt varies 2..14, so pure data-parallel wastes up to 2x on
    the max core.  We instead split attention into 4-key-tile "windows"
    (512 key cols) and give every core exactly WPC windows x all 2048
    queries; host sums the partial (numerator|denominator) outputs.
  * The conv3/conv5 + concat + K-projection fold into 5 shifted matmuls
    (taps), done in fp8 DoubleRow pairs (10 pairs of k-tiles).
  * Residual V must be fp16-accurate; VT is computed fp16 per 512-col
    chunk, 4 chunks/core (32 total = exactly one copy of V). Windows are
    co-located with the VT chunk covering their key columns so V1 (fp8,
    transposed, [V|1|0-pad] to M=112) derives locally.
  * Masking: per (window-tile) per-partition bias 0/-1e6 into the exp
    ACT reproduces the reference where()+softmax exactly (exp(-1e6+s)
    underflows to 0).
"""

import os
import numpy as np
import ml_dtypes

import concourse.bacc as bacc
import concourse.bass as bass
import concourse.tile as tile
from concourse import mybir
from concourse._compat import get_trn_type
from concourse.bass_utils import run_bass_kernel_spmd

B, L = 8, 2048
Q_IN, V_IN, QK, VD = 512, 1024, 96, 96
P = 128
CW = 512          # column chunk width (= PSUM bank of f32)
WT = 4            # key tiles per window
NCHUNK = L // CW  # 4 column chunks per sequence
NORM = float(1.0 / np.sqrt(QK))
F32 = mybir.dt.float32
F16 = mybir.dt.float16
F8 = mybir.dt.float8e4
NP8 = ml_dtypes.float8_e4m3
DR = mybir.MatmulPerfMode.DoubleRow
EXP = mybir.ActivationFunctionType.Exp

LAST_EXEC_TIME_NS = None
LAST_RESULTS = None

_program_cache = {}


def _fold_k_weights(k_w, k_b, cn3_w, cn3_b, cn5_w, cn5_b):
    """K[l] = sum_{t in -2..2} evo[l+t] @ taps[t+2] + bk  (zero-padded shifts)."""
    A_evo = k_w[:, :Q_IN]
    A3 = k_w[:, Q_IN : Q_IN + VD]
    A5 = k_w[:, Q_IN + VD :]
    taps = np.zeros((5, Q_IN, QK), np.float32)
    for j in range(3):  # conv3 tap j acts at offset t = j-1
        taps[j - 1 + 2] += np.einsum("oc,cd->do", A3, cn3_w[:, :, j]).astype(np.float32)
    for j in range(5):  # conv5 tap j acts at offset t = j-2
        taps[j - 2 + 2] += np.einsum("oc,cd->do", A5, cn5_w[:, :, j]).astype(np.float32)
    taps[2] += A_evo.T
    bk = (k_b + A3 @ cn3_b + A5 @ cn5_b).astype(np.float32)
    return taps, bk


def _build_program(wpc):
    """One SPMD program; every core runs `wpc` attention windows (4 key
    tiles x 2048 queries each) + 4 fp16 V-projection chunks."""
    nc = bacc.Bacc(get_trn_type() or "TRN2", target_bir_lowering=False, debug=False)
    # weights / constants
    wq = nc.declare_dram_parameter("wq", [P, 2 * 2 * P], F8, isOutput=False)
    wk = nc.declare_dram_parameter("wk", [P, 5 * 2 * 2 * P], F8, isOutput=False)
    wv = nc.declare_dram_parameter("wv", [P, 8 * QK], F16, isOutput=False)
    bqk = nc.declare_dram_parameter("bqk", [P, 2], F32, isOutput=False)  # padded q_b | k_b
    bv = nc.declare_dram_parameter("bv", [QK, 1], F32, isOutput=False)
    maskd = nc.declare_dram_parameter("mask", [P, wpc * WT], F32, isOutput=False)
    identd = nc.declare_dram_parameter("ident", [QK, QK], F16, isOutput=False)
    # activations: per window evo (halo slice for K, full for Q), per chunk plm
    evoW = [
        nc.declare_dram_parameter(f"evoW{w}", [P, 4 * (CW + 4)], F8, isOutput=False)
        for w in range(wpc)
    ]
    evoQ = [
        nc.declare_dram_parameter(f"evoQ{w}", [P, 4 * L], F8, isOutput=False)
        for w in range(wpc)
    ]
    plm = [
        nc.declare_dram_parameter(f"plm{r}", [P, 8 * CW], F16, isOutput=False)
        for r in range(NCHUNK)
    ]
    # outputs: per window OT partial (96 rows numerator + 1 row denominator),
    # per chunk vt (fp16 V for the residual)
    ot_out = [
        nc.declare_dram_parameter(f"ot{w}", [QK + 1, L], F32, isOutput=True)
        for w in range(wpc)
    ]
    vt_out = [
        nc.declare_dram_parameter(f"vt{r}", [QK, CW], F16, isOutput=True)
        for r in range(NCHUNK)
    ]

    add = mybir.AluOpType.add

    with tile.TileContext(nc) as tc:
        with tc.tile_pool(name="sing", bufs=1) as sing:
            # ---- small weights first (gate the first matmuls) ----
            wq_sb = sing.tile([P, 2, 2, P], F8, tag="wq")
            nc.sync.dma_start(
                out=wq_sb, in_=wq[:, :].rearrange("p (g i m) -> p g i m", g=2, i=2)
            )
            wk_sb = sing.tile([P, 5, 2, 2, P], F8, tag="wk")
            nc.sync.dma_start(
                out=wk_sb, in_=wk[:, :].rearrange("p (t g i m) -> p t g i m", t=5, g=2, i=2)
            )
            bqk_sb = sing.tile([P, 2], F32, tag="bqk")
            nc.sync.dma_start(out=bqk_sb, in_=bqk[:, :])
            mask_sb = sing.tile([P, wpc * WT], F32, tag="mask")
            nc.sync.dma_start(out=mask_sb, in_=maskd[:, :])
            # evo for K windows (tiny, needed first): pair tiles (dt01, dt23)
            evw_sb = []
            for w in range(wpc):
                pair = []
                for g in range(2):
                    t = sing.tile([P, 2, CW + 4], F8, tag=f"evw{w}_{g}", name=f"evw{w}_{g}")
                    nc.sync.dma_start(
                        out=t,
                        in_=evoW[w][:, :]
                        .rearrange("p (g i c) -> p g i c", g=2, i=2)[:, g],
                    )
                    pair.append(t)
                evw_sb.append(pair)
            # evo for Q (full length)
            evq_sb = []
            for w in range(wpc):
                pair = []
                for g in range(2):
                    t = sing.tile([P, 2, L], F8, tag=f"evq{w}_{g}", name=f"evq{w}_{g}")
                    for h in range(2):
                        nc.sync.dma_start(
                            out=t[h * 64 : (h + 1) * 64],
                            in_=evoQ[w][h * 64 : (h + 1) * 64, :]
                            .rearrange("p (g i c) -> p g i c", g=2, i=2)[:, g],
                        )
                    pair.append(t)
                evq_sb.append(pair)
            wv_sb = sing.tile([P, 8, QK], F16, tag="wv")
            nc.sync.dma_start(out=wv_sb, in_=wv[:, :].rearrange("p (n o) -> p n o", o=QK))
            bv_sb = sing.tile([QK, 1], F32, tag="bv")
            nc.sync.dma_start(out=bv_sb, in_=bv[:, :])
            ident_sb = sing.tile([QK, QK], F16, tag="ident")
            nc.sync.dma_start(out=ident_sb, in_=identd[:, :])
            plm_sb = []
            for r in range(NCHUNK):
                t = sing.tile([P, 8, CW], F16, tag=f"plm{r}", name=f"plmsb{r}")
                for h in range(4):
                    nc.sync.dma_start(
                        out=t[:, 2 * h : 2 * h + 2, :],
                        in_=plm[r][:, :].rearrange("p (n c) -> p n c", n=8)[
                            :, 2 * h : 2 * h + 2
                        ],
                    )
                plm_sb.append(t)

            kt_sb = [sing.tile([64, 2, CW], F8, tag=f"kt{w}", name=f"kt{w}") for w in range(wpc)]
            qt_sb = [sing.tile([64, 2, L], F8, tag=f"qt{w}", name=f"qt{w}") for w in range(wpc)]
            vt_sb = [sing.tile([QK, CW], F16, tag=f"vt{r}", name=f"vt{r}") for r in range(NCHUNK)]
            v1_sb = [sing.tile([P, 2, 2, 112], F8, tag=f"v1_{w}", name=f"v1_{w}") for w in range(wpc)]
            ot_sb = [sing.tile([QK + 1, L], F32, tag=f"ot{w}", name=f"ot{w}") for w in range(wpc)]

            for w in range(wpc):
                nc.vector.memset(v1_sb[w], 0.0)

            # ---- projections ----
            with (
                tc.tile_pool(name="kq_psum", bufs=3, space="PSUM") as kq_psum,
                tc.tile_pool(name="v_psum", bufs=2, space="PSUM") as v_psum,
                tc.tile_pool(name="t_psum", bufs=2, space="PSUM") as t_psum,
            ):
                # KT windows: 10 DoubleRow (tap x dt-pair) accumulations
                for w in range(wpc):
                    pt = kq_psum.tile([P, CW], F32, tag="kq")
                    n = 0
                    for t in range(5):
                        for g in range(2):
                            nc.tensor.matmul(
                                pt,
                                lhsT=wk_sb[:, t, g],
                                rhs=evw_sb[w][g][:, :, t : t + CW],
                                start=(n == 0),
                                stop=(n == 9),
                                perf_mode=DR,
                            )
                            n += 1
                    for i in range(2):
                        nc.vector.tensor_scalar(
                            out=kt_sb[w][:, i, :],
                            in0=pt[64 * i : 64 * (i + 1), :],
                            scalar1=bqk_sb[64 * i : 64 * (i + 1), 1:2],
                            scalar2=None,
                            op0=add,
                        )
                # QT windows (full L, 512-col chunks)
                for w in range(wpc):
                    for c in range(NCHUNK):
                        pt = kq_psum.tile([P, CW], F32, tag="kq")
                        for g in range(2):
                            nc.tensor.matmul(
                                pt,
                                lhsT=wq_sb[:, g],
                                rhs=evq_sb[w][g][:, :, c * CW : (c + 1) * CW],
                                start=(g == 0),
                                stop=(g == 1),
                                perf_mode=DR,
                            )
                        for i in range(2):
                            nc.vector.tensor_scalar(
                                out=qt_sb[w][:, i, c * CW : (c + 1) * CW],
                                in0=pt[64 * i : 64 * (i + 1), :],
                                scalar1=bqk_sb[64 * i : 64 * (i + 1), 0:1],
                                scalar2=None,
                                op0=add,
                            )
                # VT chunks (fp16, residual-accurate; chunks 0..wpc-1 feed V1)
                for r in range(NCHUNK):
                    pt = v_psum.tile([QK, CW], F32, tag="v")
                    for dt in range(8):
                        nc.tensor.matmul(
                            pt,
                            lhsT=wv_sb[:, dt],
                            rhs=plm_sb[r][:, dt],
                            start=(dt == 0),
                            stop=(dt == 7),
                        )
                    nc.vector.tensor_scalar(
                        out=vt_sb[r],
                        in0=pt,
                        scalar1=bv_sb[:, 0:1],
                        scalar2=None,
                        op0=add,
                    )
                    nc.sync.dma_start(out=vt_out[r][:, :], in_=vt_sb[r])
                    if r < wpc:
                        # V1 for window r: PE transpose (fp16) + fp8 cast copy
                        for j in range(WT):
                            vp = t_psum.tile([P, QK], F16, tag="t")
                            nc.tensor.transpose(
                                vp, vt_sb[r][:, j * P : (j + 1) * P], ident_sb
                            )
                            nc.vector.tensor_copy(
                                out=v1_sb[r][:, j // 2, j % 2, :QK], in_=vp
                            )
                        nc.vector.memset(v1_sb[r][:, 0, 0, QK : QK + 1], 1.0)
                        nc.vector.memset(v1_sb[r][:, 0, 1, QK : QK + 1], 1.0)
                        nc.vector.memset(v1_sb[r][:, 1, 0, QK : QK + 1], 1.0)
                        nc.vector.memset(v1_sb[r][:, 1, 1, QK : QK + 1], 1.0)

            # ---- attention windows ----
            with (
                tc.tile_pool(name="st_psum", bufs=2, space="PSUM") as st_psum,
                tc.tile_pool(name="ot_psum", bufs=1, space="PSUM") as ot_psum,
                tc.tile_pool(name="et", bufs=2) as et_pool,
            ):
                for w in range(wpc):
                    et8 = et_pool.tile([P, 2, 2, L], F8, tag="et")
                    for j in range(WT):
                        for h in range(2):
                            stp = st_psum.tile([P, 2 * CW], F32, tag="st")
                            for o in range(2):
                                nc.tensor.matmul(
                                    stp[:, o * CW : (o + 1) * CW],
                                    lhsT=kt_sb[w][:, :, j * P : (j + 1) * P],
                                    rhs=qt_sb[w][
                                        :, :, (2 * h + o) * CW : (2 * h + o + 1) * CW
                                    ],
                                    start=True,
                                    stop=True,
                                    perf_mode=DR,
                                )
                            nc.scalar.activation(
                                out=et8[:, j // 2, j % 2, h * 2 * CW : (h + 1) * 2 * CW],
                                in_=stp,
                                func=EXP,
                                bias=mask_sb[:, w * WT + j : w * WT + j + 1],
                                scale=NORM,
                            )
                    otp = ot_psum.tile([112, L], F32, tag="ot")
                    for p in range(2):
                        for c in range(NCHUNK):
                            nc.tensor.matmul(
                                otp[:, c * CW : (c + 1) * CW],
                                lhsT=v1_sb[w][:, p],
                                rhs=et8[:, p, :, c * CW : (c + 1) * CW],
                                start=(p == 0),
                                stop=(p == 1),
                                perf_mode=DR,
                            )
                    nc.vector.tensor_copy(out=ot_sb[w], in_=otp[: QK + 1, :])
                    for c in range(NCHUNK):
                        nc.sync.dma_start(
                            out=ot_out[w][:QK, c * CW : (c + 1) * CW],
                            in_=ot_sb[w][:QK, c * CW : (c + 1) * CW],
                        )
                        nc.scalar.dma_start(
                            out=ot_out[w][QK : QK + 1, c * CW : (c + 1) * CW],
                            in_=ot_sb[w][QK : QK + 1, c * CW : (c + 1) * CW],
                        )
    nc.finalize()
    return nc


def _pack_pair_w(w, nk):
    """(nk*128, M) f32 -> [128, nk/2, 2, M] DoubleRow pair layout."""
    kt, m = nk, w.shape[1]
    v = w.reshape(kt, P, m).reshape(kt // 2, 2, P, m).transpose(2, 0, 1, 3)
    return np.ascontiguousarray(v)  # [P, nk/2, 2, M]


def _plan(seqlengths):
    """Assign windows + residual chunks to cores.

    Returns wpc, and per-core lists: windows[c] = [(b, w) or None], and
    chunks[c] = [(b, r)] of length NCHUNK with chunks[c][i] = window i's
    key-column chunk for i < len(real windows)."""
    nkt = [max(1, min(L // P, -(-int(s) // P))) for s in seqlengths]
    wins = [(b, w) for b in range(B) for w in range(-(-nkt[b] // WT))]
    wpc = max(1, -(-len(wins) // B))
    # pad with None (dummy) windows
    wins = wins + [None] * (B * wpc - len(wins))
    windows = [wins[c * wpc : (c + 1) * wpc] for c in range(B)]
    # chunk assignment: R slot w must hold window w's key-column chunk
    # (V1 derives from that VT locally); remaining slots take leftovers.
    all_chunks = {(b, r) for b in range(B) for r in range(NCHUNK)}
    chunks = [[None] * NCHUNK for _ in range(B)]
    for c in range(B):
        for w, bw in enumerate(windows[c]):
            if bw is not None:
                assert bw in all_chunks
                chunks[c][w] = bw
                all_chunks.discard(bw)
    rest = sorted(all_chunks)
    for c in range(B):
        for r in range(NCHUNK):
            if chunks[c][r] is None:
                chunks[c][r] = rest.pop()
    assert not rest
    return wpc, nkt, windows, chunks


def _prep_core(core, wpc, nkt, windows, chunks, evoT8, plmT, seqlengths, weights):
    m = dict(weights)
    mask = np.full((P, wpc * WT), -1e6, np.float32)
    p = np.arange(P)
    for w, bw in enumerate(windows[core]):
        if bw is None:
            b0 = 0
            m[f"evoW{w}"] = np.zeros((P, 4 * (CW + 4)), NP8)
            m[f"evoQ{w}"] = np.zeros((P, 4 * L), NP8)
            continue
        b, wi = bw
        sl = int(seqlengths[b])
        base = wi * WT * P
        for j in range(WT):
            mask[:, w * WT + j] = np.where(base + j * P + p < sl, 0.0, -1e6)
        # evo slice with +-2 halo for the 5-tap K matmuls; evoT8 is
        # pre-padded by 2 on both sides (width L+4)
        sl_ = evoT8[b][:, base : base + CW + 4]
        m[f"evoW{w}"] = np.ascontiguousarray(
            sl_.reshape(4, P, CW + 4).transpose(1, 0, 2).reshape(P, -1)
        )
        m[f"evoQ{w}"] = np.ascontiguousarray(
            evoT8[b][:, 2 : 2 + L].reshape(4, P, L).transpose(1, 0, 2).reshape(P, -1)
        )
    m["mask"] = mask
    for r, (b, rc) in enumerate(chunks[core]):
        sl_ = plmT[b][:, rc * CW : (rc + 1) * CW]
        m[f"plm{r}"] = np.ascontiguousarray(
            sl_.reshape(8, P, CW).transpose(1, 0, 2).reshape(P, -1)
        )
    return m


def kernel(
    plm_embedding,
    evo_local,
    seqlengths,
    q_w,
    q_b,
    k_w,
    k_b,
    v_w,
    v_b,
    cn3_w,
    cn3_b,
    cn5_w,
    cn5_b,
):
    global LAST_EXEC_TIME_NS, LAST_RESULTS
    plm_embedding = np.asarray(plm_embedding, np.float32)
    evo_local = np.asarray(evo_local, np.float32)
    seqlengths = np.asarray(seqlengths)

    taps, bk = _fold_k_weights(
        np.asarray(k_w, np.float32),
        np.asarray(k_b, np.float32),
        np.asarray(cn3_w, np.float32),
        np.asarray(cn3_b, np.float32),
        np.asarray(cn5_w, np.float32),
        np.asarray(cn5_b, np.float32),
    )
    wpc, nkt, windows, chunks = _plan(seqlengths)

    # fp8 weights, M padded 96 -> 128 with zeros (bias pad rows also zero so
    # padded Q/K rows contribute exactly 0 to scores)
    wq_p = np.zeros((Q_IN, P), np.float32)
    wq_p[:, :QK] = np.asarray(q_w, np.float32).T
    wk_p = np.zeros((5 * Q_IN, P), np.float32)
    wk_p[:, :QK] = taps.reshape(5 * Q_IN, QK)
    bqk = np.zeros((P, 2), np.float32)
    bqk[:QK, 0] = np.asarray(q_b, np.float32)
    bqk[:QK, 1] = bk
    weights = {
        "wq": np.ascontiguousarray(_pack_pair_w(wq_p, 4).reshape(P, -1)).astype(NP8),
        "wk": np.ascontiguousarray(_pack_pair_w(wk_p, 20).reshape(P, -1)).astype(NP8),
        "wv": np.ascontiguousarray(
            np.asarray(v_w, np.float32)
            .T.reshape(8, P, QK)
            .transpose(1, 0, 2)
            .reshape(P, -1)
        ).astype(np.float16),
        "bqk": bqk,
        "bv": np.ascontiguousarray(np.asarray(v_b, np.float32)[:, None]),
        "ident": np.eye(QK, dtype=np.float16),
    }

    evoT8 = np.zeros((B, Q_IN, L + 4), NP8)
    evoT8[:, :, 2 : 2 + L] = np.clip(
        evo_local.transpose(0, 2, 1), -240.0, 240.0
    ).astype(NP8)
    plmT = plm_embedding.transpose(0, 2, 1).astype(np.float16)

    if wpc not in _program_cache:
        _program_cache[wpc] = _build_program(wpc)
    nc = _program_cache[wpc]

    in_maps = [
        _prep_core(c, wpc, nkt, windows, chunks, evoT8, plmT, seqlengths, weights)
        for c in range(B)
    ]
    trace = bool(os.environ.get("KBENCH_TRACE"))
    res = run_bass_kernel_spmd(nc, in_maps, list(range(B)), trace=trace)
    LAST_EXEC_TIME_NS = res.exec_time_ns
    LAST_RESULTS = res

    # host gather: sum window partials per sequence, divide, add residual
    num = np.zeros((B, QK, L), np.float32)
    den = np.zeros((B, 1, L), np.float32)
    vt = np.zeros((B, QK, L), np.float32)
    for c in range(B):
        for w, bw in enumerate(windows[c]):
            if bw is None:
                continue
            b, _ = bw
            ot = res.results[c][f"ot{w}"]
            num[b] += ot[:QK]
            den[b] += ot[QK : QK + 1]
        for r, (b, rc) in enumerate(chunks[c]):
            vt[b][:, rc * CW : (rc + 1) * CW] = res.results[c][f"vt{r}"]
    out = ((num / den) + vt).transpose(0, 2, 1).astype(np.float32)
    return np.ascontiguousarray(out)


# revision 9
# speedup vs baseline: 1.1551x; 1.0741x over previous
"""Contextual-attention Trainium2 kernel (Bass/Tile), work-balanced across cores.

Math (per sequence b):
    Q = evo @ q_w.T + q_b                                  (L, 96)
    K = cat(evo, conv3(evo), conv5(evo)) @ k_w.T + k_b     (L, 96)
    V = plm @ v_w.T + v_b                                  (L, 96)
    P = softmax(Q K^T / sqrt(96), key-masked by seqlen)
    out = P @ V + V

Design notes (measured on hw):
  * PE streams 1 output column/cycle (~1.35 GHz) regardless of dtype; fp8
    DoubleRow processes 2 k-tiles per pass -> 2x for pairable contractions
    (QT/KT/OT). The 96-dim score contraction is not pairable beyond a
    zero-padded 2x64 split, so ST cost is fixed; balancing it across cores
    is what wins.
  * Work per sequence scales with nkt_b = ceil(seqlen_b/128) key tiles
    (2..14 here). Attention is split into 4-key-tile "windows" (512 key
    cols x all 2048 queries); every core gets exactly wpc windows + 4 fp16
    V chunks -> perfectly uniform SPMD program; host sums the partial
    (numerator|denominator) window outputs.
  * The PE drops to a ~1.66x slower p-state after any blocking wait and
    needs ~3us of uninterrupted issue to recover, so the emission order
    keeps every PE instruction's deps satisfied ahead of time: VT chunks
    2..3 are interleaved as filler between ACT-bound score units, OT
    drains per half-window, DMA issue is spread across sync/gpsimd
    queues, and the scalar queue carries nothing but the exp ACTs.
  * Masking: per (window-tile) per-partition bias 0/-1e6 into the exp ACT
    reproduces the reference where()+softmax exactly (exp(-1e6+s) == 0).
"""

import os
import numpy as np
import ml_dtypes

import concourse.bacc as bacc
import concourse.bass as bass
import concourse.tile as tile
from concourse import mybir
from concourse._compat import get_trn_type
from concourse.bass_utils import run_bass_kernel_spmd

B, L = 8, 2048
Q_IN, V_IN, QK, VD = 512, 1024, 96, 96
P = 128
CW = 512          # column chunk width (= one PSUM bank of f32)
WT = 4            # key tiles per window
NCHUNK = L // CW  # 4 column chunks per sequence
NORM = float(1.0 / np.sqrt(QK))
F32 = mybir.dt.float32
F16 = mybir.dt.float16
F8 = mybir.dt.float8e4
NP8 = ml_dtypes.float8_e4m3
DR = mybir.MatmulPerfMode.DoubleRow
EXP = mybir.ActivationFunctionType.Exp

LAST_EXEC_TIME_NS = None
LAST_RESULTS = None

_program_cache = {}


def _fold_k_weights(k_w, k_b, cn3_w, cn3_b, cn5_w, cn5_b):
    """K[l] = sum_{t in -2..2} evo[l+t] @ taps[t+2] + bk  (zero-padded shifts)."""
    A_evo = k_w[:, :Q_IN]
    A3 = k_w[:, Q_IN : Q_IN + VD]
    A5 = k_w[:, Q_IN + VD :]
    taps = np.zeros((5, Q_IN, QK), np.float32)
    for j in range(3):  # conv3 tap j acts at offset t = j-1
        taps[j - 1 + 2] += np.einsum("oc,cd->do", A3, cn3_w[:, :, j]).astype(np.float32)
    for j in range(5):  # conv5 tap j acts at offset t = j-2
        taps[j - 2 + 2] += np.einsum("oc,cd->do", A5, cn5_w[:, :, j]).astype(np.float32)
    taps[2] += A_evo.T
    bk = (k_b + A3 @ cn3_b + A5 @ cn5_b).astype(np.float32)
    return taps, bk


def _build_program(wpc):
    nc = bacc.Bacc(get_trn_type() or "TRN2", target_bir_lowering=False, debug=False)
    wq = nc.declare_dram_parameter("wq", [P, 2 * 2 * P], F8, isOutput=False)
    wk = nc.declare_dram_parameter("wk", [P, 5 * 2 * 2 * P], F8, isOutput=False)
    wv = nc.declare_dram_parameter("wv", [P, 8 * QK], F16, isOutput=False)
    bqk = nc.declare_dram_parameter("bqk", [P, 2], F32, isOutput=False)
    bv = nc.declare_dram_parameter("bv", [QK, 1], F32, isOutput=False)
    maskd = nc.declare_dram_parameter("mask", [P, wpc * WT], F32, isOutput=False)
    identd = nc.declare_dram_parameter("ident", [QK, QK], F16, isOutput=False)
    evoW = [
        nc.declare_dram_parameter(f"evoW{w}", [P, 4 * (CW + 4)], F8, isOutput=False)
        for w in range(wpc)
    ]
    evoQ = [
        nc.declare_dram_parameter(f"evoQ{w}", [P, 4 * L], F8, isOutput=False)
        for w in range(wpc)
    ]
    plm = [
        nc.declare_dram_parameter(f"plm{r}", [P, 8 * CW], F16, isOutput=False)
        for r in range(NCHUNK)
    ]
    ot_out = [
        nc.declare_dram_parameter(f"ot{w}", [QK + 1, L], F32, isOutput=True)
        for w in range(wpc)
    ]
    vt_out = [
        nc.declare_dram_parameter(f"vt{r}", [QK, CW], F16, isOutput=True)
        for r in range(NCHUNK)
    ]

    add = mybir.AluOpType.add

    with tile.TileContext(nc) as tc:
        with tc.tile_pool(name="sing", bufs=1) as sing:
            # ---- SBUF tiles ----
            wq_sb = sing.tile([P, 2, 2, P], F8, tag="wq")
            wk_sb = sing.tile([P, 5, 2, 2, P], F8, tag="wk")
            wv_sb = sing.tile([P, 8, QK], F16, tag="wv")
            bqk_sb = sing.tile([P, 2], F32, tag="bqk")
            bv_sb = sing.tile([QK, 1], F32, tag="bv")
            mask_sb = sing.tile([P, wpc * WT], F32, tag="mask")
            ident_sb = sing.tile([QK, QK], F16, tag="ident")
            evw_sb = [
                [sing.tile([P, 2, CW + 4], F8, tag=f"evw{w}_{g}", name=f"evw{w}_{g}")
                 for g in range(2)]
                for w in range(wpc)
            ]
            evq_sb = [
                [sing.tile([P, 2, L], F8, tag=f"evq{w}_{g}", name=f"evq{w}_{g}")
                 for g in range(2)]
                for w in range(wpc)
            ]
            plm_sb = [
                sing.tile([P, 8, CW], F16, tag=f"plm{r}", name=f"plmsb{r}")
                for r in range(NCHUNK)
            ]
            kt_sb = [sing.tile([64, 2, CW], F8, tag=f"kt{w}", name=f"kt{w}") for w in range(wpc)]
            qt_sb = [sing.tile([64, 2, L], F8, tag=f"qt{w}", name=f"qt{w}") for w in range(wpc)]
            vt_sb = [sing.tile([QK, CW], F16, tag=f"vt{r}", name=f"vt{r}") for r in range(NCHUNK)]
            v1_sb = [sing.tile([P, 2, 2, 112], F8, tag=f"v1_{w}", name=f"v1_{w}") for w in range(wpc)]
            et_sb = [sing.tile([P, 2, 2, L], F8, tag=f"et{w}", name=f"et{w}") for w in range(wpc)]
            ot_sb = [sing.tile([QK + 1, L], F32, tag=f"ot{w}", name=f"ot{w}") for w in range(wpc)]

            # ---- DMA issue: sync queue = weights + evo; gpsimd = plm ----
            nc.sync.dma_start(
                out=wq_sb, in_=wq[:, :].rearrange("p (g i m) -> p g i m", g=2, i=2)
            )
            wk_r = wk[:, :].rearrange("p (t g i m) -> p t g i m", t=5, g=2, i=2)
            for t4 in range(5):
                nc.sync.dma_start(out=wk_sb[:, t4], in_=wk_r[:, t4])
            for w in range(wpc):
                for g in range(2):
                    nc.sync.dma_start(
                        out=evw_sb[w][g],
                        in_=evoW[w][:, :].rearrange("p (g i c) -> p g i c", g=2, i=2)[:, g],
                    )
            nc.sync.dma_start(out=bqk_sb, in_=bqk[:, :])
            nc.sync.dma_start(out=mask_sb, in_=maskd[:, :])
            for r in range(NCHUNK):
                for h in range(4):
                    nc.gpsimd.dma_start(
                        out=plm_sb[r][:, 2 * h : 2 * h + 2, :],
                        in_=plm[r][:, :].rearrange("p (n c) -> p n c", n=8)[
                            :, 2 * h : 2 * h + 2
                        ],
                    )
            for w in range(wpc):
                for g in range(2):
                    for h in range(2):
                        nc.sync.dma_start(
                            out=evq_sb[w][g][h * 64 : (h + 1) * 64],
                            in_=evoQ[w][h * 64 : (h + 1) * 64, :]
                            .rearrange("p (g i c) -> p g i c", g=2, i=2)[:, g],
                        )
            nc.sync.dma_start(
                out=wv_sb, in_=wv[:, :].rearrange("p (n o) -> p n o", o=QK)
            )
            nc.sync.dma_start(out=bv_sb, in_=bv[:, :])
            nc.sync.dma_start(out=ident_sb, in_=identd[:, :])

            for w in range(wpc):
                nc.vector.memset(v1_sb[w], 0.0)
                for p in range(2):
                    for i in range(2):
                        nc.vector.memset(v1_sb[w][:, p, i, QK : QK + 1], 1.0)

            IDENT = mybir.ActivationFunctionType.Identity

            def kt_split(w, pt):
                nc.vector.tensor_scalar(
                    out=kt_sb[w][:, 0, :], in0=pt[0:64, :],
                    scalar1=bqk_sb[0:64, 1:2], scalar2=None, op0=add,
                )
                nc.scalar.activation(
                    out=kt_sb[w][:, 1, :], in_=pt[64:128, :],
                    func=IDENT, bias=bqk_sb[64:128, 1:2], scale=1.0,
                )

            def qt_split(w, c, pt):
                nc.vector.tensor_scalar(
                    out=qt_sb[w][:, 0, c * CW : (c + 1) * CW], in0=pt[0:64, :],
                    scalar1=bqk_sb[0:64, 0:1], scalar2=None, op0=add,
                )
                nc.scalar.activation(
                    out=qt_sb[w][:, 1, c * CW : (c + 1) * CW], in_=pt[64:128, :],
                    func=IDENT, bias=bqk_sb[64:128, 0:1], scale=1.0,
                )

            with tc.tile_pool(name="v_psum", bufs=2, space="PSUM") as v_psum:

                def vt_head(r):
                    # returns the psum tile; 8 accumulation matmuls issued via vt_mm
                    return v_psum.tile([QK, CW], F32, tag="v", name=f"vtp{r}")

                def vt_mm(pt, r, dt):
                    nc.tensor.matmul(
                        pt, lhsT=wv_sb[:, dt], rhs=plm_sb[r][:, dt],
                        start=(dt == 0), stop=(dt == 7),
                    )

                def vt_drain(pt, r):
                    nc.vector.tensor_scalar(
                        out=vt_sb[r], in0=pt, scalar1=bv_sb[:, 0:1],
                        scalar2=None, op0=add,
                    )
                    nc.gpsimd.dma_start(out=vt_out[r][:, :], in_=vt_sb[r])

                # ---- projections: KT, QT (DoubleRow fp8), VT chunks 0..wpc-1 ----
                with (
                    tc.tile_pool(name="kq_psum", bufs=4, space="PSUM") as kq_psum,
                    tc.tile_pool(name="t_psum", bufs=2, space="PSUM") as t_psum,
                ):
                    for w in range(wpc):
                        pt = kq_psum.tile([P, CW], F32, tag="kq")
                        n = 0
                        for t in range(5):
                            for g in range(2):
                                nc.tensor.matmul(
                                    pt,
                                    lhsT=wk_sb[:, t, g],
                                    rhs=evw_sb[w][g][:, :, t : t + CW],
                                    start=(n == 0),
                                    stop=(n == 9),
                                    perf_mode=DR,
                                )
                                n += 1
                        kt_split(w, pt)
                    for w in range(wpc):
                        for c in range(NCHUNK):
                            pt = kq_psum.tile([P, CW], F32, tag="kq")
                            for g in range(2):
                                nc.tensor.matmul(
                                    pt,
                                    lhsT=wq_sb[:, g],
                                    rhs=evq_sb[w][g][:, :, c * CW : (c + 1) * CW],
                                    start=(g == 0),
                                    stop=(g == 1),
                                    perf_mode=DR,
                                )
                            qt_split(w, c, pt)
                    # VT chunks 0..wpc-1 (feed V1) + their transposes
                    for r in range(min(wpc, NCHUNK)):
                        pt = vt_head(r)
                        for dt in range(8):
                            vt_mm(pt, r, dt)
                        vt_drain(pt, r)
                        for j in range(WT):
                            vp = t_psum.tile([P, QK], F16, tag="t")
                            nc.tensor.transpose(
                                vp, vt_sb[r][:, j * P : (j + 1) * P], ident_sb
                            )
                            nc.vector.tensor_copy(
                                out=v1_sb[r][:, j // 2, j % 2, :QK], in_=vp
                            )

                # ---- attention: score units interleaved with VT fills ----
                fills = []
                for r in range(wpc, NCHUNK):
                    holder = {}

                    def mk(r, dt, holder=None):
                        def run():
                            if dt == 0:
                                holder["pt"] = vt_head(r)
                            vt_mm(holder["pt"], r, dt)
                            if dt == 7:
                                vt_drain(holder["pt"], r)
                        return run

                    for dt in range(8):
                        fills.append(mk(r, dt, holder))

                fi = 0

                def fill_one():
                    nonlocal fi
                    if fi < len(fills):
                        fills[fi]()
                        fi += 1

                with (
                    tc.tile_pool(name="st_psum", bufs=2, space="PSUM") as st_psum,
                    tc.tile_pool(name="ot_psum", bufs=1, space="PSUM") as ot_psum,
                ):

                    def aunit(w, j, h):
                        stp = st_psum.tile([P, 2 * CW], F32, tag="st")
                        for o in range(2):
                            nc.tensor.matmul(
                                stp[:, o * CW : (o + 1) * CW],
                                lhsT=kt_sb[w][:, :, j * P : (j + 1) * P],
                                rhs=qt_sb[w][
                                    :, :, (2 * h + o) * CW : (2 * h + o + 1) * CW
                                ],
                                start=True,
                                stop=True,
                                perf_mode=DR,
                            )
                        nc.scalar.activation(
                            out=et_sb[w][:, j // 2, j % 2, h * 2 * CW : (h + 1) * 2 * CW],
                            in_=stp,
                            func=EXP,
                            bias=mask_sb[:, w * WT + j : w * WT + j + 1],
                            scale=NORM,
                        )

                    def otgroup(w, h):
                        otp = ot_psum.tile([112, 2 * CW], F32, tag="ot")
                        for p in range(2):
                            for o in range(2):
                                c = 2 * h + o
                                nc.tensor.matmul(
                                    otp[:, o * CW : (o + 1) * CW],
                                    lhsT=v1_sb[w][:, p],
                                    rhs=et_sb[w][:, p, :, c * CW : (c + 1) * CW],
                                    start=(p == 0),
                                    stop=(p == 1),
                                    perf_mode=DR,
                                )
                        nc.vector.tensor_copy(
                            out=ot_sb[w][:, h * 2 * CW : (h + 1) * 2 * CW],
                            in_=otp[: QK + 1, :],
                        )
                        nc.sync.dma_start(
                            out=ot_out[w][:QK, h * 2 * CW : (h + 1) * 2 * CW],
                            in_=ot_sb[w][:QK, h * 2 * CW : (h + 1) * 2 * CW],
                        )
                        nc.gpsimd.dma_start(
                            out=ot_out[w][QK : QK + 1, h * 2 * CW : (h + 1) * 2 * CW],
                            in_=ot_sb[w][QK : QK + 1, h * 2 * CW : (h + 1) * 2 * CW],
                        )

                    # emission: A-units of window w, then OT(w,h0); OT(w,h1)
                    # rides 4 A-units into the next window (deps long done)
                    pending = []
                    for w in range(wpc):
                        cnt = 0
                        for j in range(WT):
                            for h in range(2):
                                aunit(w, j, h)
                                fill_one()
                                cnt += 1
                                if cnt == 4 and pending:
                                    otgroup(*pending.pop(0))
                        otgroup(w, 0)
                        pending.append((w, 1))
                    for wh in pending:
                        otgroup(*wh)
    nc.finalize()
    return nc


def _pack_pair_w(w, nk):
    """(nk*128, M) f32 -> [128, nk/2, 2, M] DoubleRow pair layout."""
    kt, m = nk, w.shape[1]
    v = w.reshape(kt, P, m).reshape(kt // 2, 2, P, m).transpose(2, 0, 1, 3)
    return np.ascontiguousarray(v)


def _plan(seqlengths):
    """Assign windows + residual chunks to cores."""
    nkt = [max(1, min(L // P, -(-int(s) // P))) for s in seqlengths]
    wins = [(b, w) for b in range(B) for w in range(-(-nkt[b] // WT))]
    wpc = max(1, -(-len(wins) // B))
    wins = wins + [None] * (B * wpc - len(wins))
    windows = [wins[c * wpc : (c + 1) * wpc] for c in range(B)]
    # R slot w must hold window w's key-column chunk (V1 derives locally)
    all_chunks = {(b, r) for b in range(B) for r in range(NCHUNK)}
    chunks = [[None] * NCHUNK for _ in range(B)]
    for c in range(B):
        for w, bw in enumerate(windows[c]):
            if w < NCHUNK and bw is not None:
                assert bw in all_chunks
                chunks[c][w] = bw
                all_chunks.discard(bw)
    rest = sorted(all_chunks)
    for c in range(B):
        for r in range(NCHUNK):
            if chunks[c][r] is None:
                chunks[c][r] = rest.pop()
    assert not rest
    return wpc, nkt, windows, chunks


def _prep_core(core, wpc, nkt, windows, chunks, evoT8, plmT, seqlengths, weights):
    m = dict(weights)
    mask = np.full((P, wpc * WT), -1e6, np.float32)
    p = np.arange(P)
    for w, bw in enumerate(windows[core]):
        if bw is None:
            m[f"evoW{w}"] = np.zeros((P, 4 * (CW + 4)), NP8)
            m[f"evoQ{w}"] = np.zeros((P, 4 * L), NP8)
            continue
        b, wi = bw
        sl = int(seqlengths[b])
        base = wi * WT * P
        for j in range(WT):
            mask[:, w * WT + j] = np.where(base + j * P + p < sl, 0.0, -1e6)
        sl_ = evoT8[b][:, base : base + CW + 4]
        m[f"evoW{w}"] = np.ascontiguousarray(
            sl_.reshape(4, P, CW + 4).transpose(1, 0, 2).reshape(P, -1)
        )
        m[f"evoQ{w}"] = np.ascontiguousarray(
            evoT8[b][:, 2 : 2 + L].reshape(4, P, L).transpose(1, 0, 2).reshape(P, -1)
        )
    m["mask"] = mask
    for r, (b, rc) in enumerate(chunks[core]):
        sl_ = plmT[b][:, rc * CW : (rc + 1) * CW]
        m[f"plm{r}"] = np.ascontiguousarray(
            sl_.reshape(8, P, CW).transpose(1, 0, 2).reshape(P, -1)
        )
    return m


def kernel(
    plm_embedding,
    evo_local,
    seqlengths,
    q_w,
    q_b,
    k_w,
    k_b,
    v_w,
    v_b,
    cn3_w,
    cn3_b,
    cn5_w,
    cn5_b,
):
    global LAST_EXEC_TIME_NS, LAST_RESULTS
    plm_embedding = np.asarray(plm_embedding, np.float32)
    evo_local = np.asarray(evo_local, np.float32)
    seqlengths = np.asarray(seqlengths)

    taps, bk = _fold_k_weights(
        np.asarray(k_w, np.float32),
        np.asarray(k_b, np.float32),
        np.asarray(cn3_w, np.float32),
        np.asarray(cn3_b, np.float32),
        np.asarray(cn5_w, np.float32),
        np.asarray(cn5_b, np.float32),
    )
    wpc, nkt, windows, chunks = _plan(seqlengths)

    # fp8 weights, M padded 96 -> 128 with zeros (pad rows of Q/K then
    # contribute exactly 0 to scores; biases pad with zeros too)
    wq_p = np.zeros((Q_IN, P), np.float32)
    wq_p[:, :QK] = np.asarray(q_w, np.float32).T
    wk_p = np.zeros((5 * Q_IN, P), np.float32)
    wk_p[:, :QK] = taps.reshape(5 * Q_IN, QK)
    bqk = np.zeros((P, 2), np.float32)
    bqk[:QK, 0] = np.asarray(q_b, np.float32)
    bqk[:QK, 1] = bk
    weights = {
        "wq": np.ascontiguousarray(_pack_pair_w(wq_p, 4).reshape(P, -1)).astype(NP8),
        "wk": np.ascontiguousarray(_pack_pair_w(wk_p, 20).reshape(P, -1)).astype(NP8),
        "wv": np.ascontiguousarray(
            np.asarray(v_w, np.float32)
            .T.reshape(8, P, QK)
            .transpose(1, 0, 2)
            .reshape(P, -1)
        ).astype(np.float16),
        "bqk": bqk,
        "bv": np.ascontiguousarray(np.asarray(v_b, np.float32)[:, None]),
        "ident": np.eye(QK, dtype=np.float16),
    }

    evoT8 = np.zeros((B, Q_IN, L + 4), NP8)
    evoT8[:, :, 2 : 2 + L] = np.clip(
        evo_local.transpose(0, 2, 1), -240.0, 240.0
    ).astype(NP8)
    plmT = plm_embedding.transpose(0, 2, 1).astype(np.float16)

    if wpc not in _program_cache:
        _program_cache[wpc] = _build_program(wpc)
    nc = _program_cache[wpc]

    in_maps = [
        _prep_core(c, wpc, nkt, windows, chunks, evoT8, plmT, seqlengths, weights)
        for c in range(B)
    ]
    trace = bool(os.environ.get("KBENCH_TRACE"))
    res = run_bass_kernel_spmd(nc, in_maps, list(range(B)), trace=trace)
    LAST_EXEC_TIME_NS = res.exec_time_ns
    LAST_RESULTS = res

    num = np.zeros((B, QK, L), np.float32)
    den = np.zeros((B, 1, L), np.float32)
    vt = np.zeros((B, QK, L), np.float32)
    for c in range(B):
        for w, bw in enumerate(windows[c]):
            if bw is None:
                continue
            b, _ = bw
            ot = res.results[c][f"ot{w}"]
            num[b] += ot[:QK]
            den[b] += ot[QK : QK + 1]
        for r, (b, rc) in enumerate(chunks[c]):
            vt[b][:, rc * CW : (rc + 1) * CW] = res.results[c][f"vt{r}"]
    out = ((num / den) + vt).transpose(0, 2, 1).astype(np.float32)
    return np.ascontiguousarray(out)
